# revision 59
# baseline (speedup 1.0000x reference)
"""Trainium2 Bass kernel for a classic Mamba block (B=2, L=2048, Dm=1024,
E=2048, N=16, R=64, K=3) running SPMD on 8 NeuronCores.

Sharding: tensor-parallel on inner dim E (E_loc = 256 per core).

Architecture: the selective scan keeps 128 e-channels in SBUF partitions
and time in the free dim; the N=16 ssm states are 16 sequential
tensor_tensor_scan tiles per (batch, e-subtile).  The n-contraction
(y = sum_n C_n*h_n) is identity-matmul PSUM accumulation on TensorE, the
D*u skip term a diag(D) matmul into the same bank, the causal depthwise
conv 3 diag(w) matmuls, conv and gating nonlinearities single Silu
activations.  The DVE is the critical engine (64 scans at ~2 cyc/elem +
the B/C broadcast muls); everything else is scheduled around keeping its
queue fed:
  - batch-0's AllReduce runs as two token-half collectives in bf16; the
    first dispatches right after the first half's dbc, and scan(0,0) runs
    as 2 chained segments so seg 0 starts off the early half-AR.
  - batch-0's z in-projection m-groups run AFTER the AR dispatches with
    both xT halves SBUF-resident (no DMA) and DVE-side PSUM drains: they
    fill the PE bubble under the AR without touching the Scalar exp chain
    or the DRAM bandwidth the collective needs.
  - B_n/C_n rows are interleaved in the AR layout (host-side W_sel row
    permutation) so one 3D broadcast DMA fetches both rows per state.
  - delta/du prep (dt matmul + softplus via exp/ln) and the dt_low loads
    for the NEXT scan group are threaded into the current scan's
    background at positions that avoid Sync/Scalar head-of-line blocks on
    AR-gated DMAs; only the PSUM y drain is emitted synchronously at each
    group boundary (it is the sole reader of the single rotating 4-bank
    PSUM y buffer - a reader emitted after the next group's first matmul
    is a race).
  - batch-0's out-proj + quarter ReduceScatters ride the scan background;
    batch-1's tail runs as two halves with the second half's drain and
    out-proj overlapping the first half's ReduceScatter.  Collectives are
    never overlapped with scans beyond this: an active collective slows
    concurrent broadcast DMAs enough to starve the scan pipeline.
"""

import sys

if "/opt/trn_rl_repo" not in sys.path:
    sys.path.insert(0, "/opt/trn_rl_repo")

from itertools import islice

import numpy as np

# ---------------------------------------------------------------------------
# Problem constants (hardcoded per contract)
B = 2
L = 2048          # sequence length per batch
DM = 1024         # model dim
E = 2048          # inner dim
N = 16            # ssm state dim
R = 64            # dt rank
K = 3             # conv kernel
N_CORES = 8
E_LOC = E // N_CORES          # 256
NS = E_LOC // 128             # e-subtiles per core (2)

FC = 512                      # psum free chunk (one bank)
FH = L // 2                   # token half

_PROGRAM_CACHE = {}


def build_program(Lb=L):
    key = Lb
    if key in _PROGRAM_CACHE:
        return _PROGRAM_CACHE[key]

    import concourse.bacc as bacc
    import concourse.mybir as mybir
    import concourse.tile as tile
    import concourse.tile_utils as tile_utils
    import concourse.bass as _bass

    if getattr(tile_utils, "max_sbuf_usage", None) is not None:
        tile_utils.max_sbuf_usage = max(tile_utils.max_sbuf_usage, 207 * 1024)

    f32 = mybir.dt.float32
    bf16 = mybir.dt.bfloat16
    f16 = mybir.dt.float16
    AF = mybir.ActivationFunctionType
    OP = mybir.AluOpType

    tok = B * Lb
    n_fc = Lb // FC               # 4 psum chunks per full-L tile
    QT = Lb // 4                  # RS quarter (tokens)
    HR = QT // N_CORES            # rows per rank per quarter

    nc = bacc.Bacc("TRN2", target_bir_lowering=False, debug=False,
                   num_devices=N_CORES)

    # ---------------- DRAM I/O ----------------
    xT = nc.dram_tensor("xT", [DM, tok], f16, kind="ExternalInput")
    w_inT = nc.dram_tensor("w_inT", [DM, 2 * E_LOC], f16, kind="ExternalInput")
    conv_diag = nc.dram_tensor("conv_diag", [128, NS * K * 128], f16,
                               kind="ExternalInput")
    conv_b = nc.dram_tensor("conv_b", [128, NS], f32, kind="ExternalInput")
    w_selT = nc.dram_tensor("w_selT", [128, NS * (R + 2 * N)], f16,
                            kind="ExternalInput")
    dt_wT = nc.dram_tensor("dt_wT", [R, E_LOC], bf16, kind="ExternalInput")
    dt_b = nc.dram_tensor("dt_b", [128, NS], f32, kind="ExternalInput")
    a_cols = nc.dram_tensor("a_cols", [128, NS * N], f32, kind="ExternalInput")
    ident = nc.dram_tensor("ident", [128, 128], bf16, kind="ExternalInput")
    d_diag = nc.dram_tensor("d_diag", [128, NS * 128], f16,
                            kind="ExternalInput")
    w_outT = nc.dram_tensor("w_outT", [128, NS * DM], f16,
                            kind="ExternalInput")

    out_loc = nc.dram_tensor("out_loc", [tok // N_CORES, DM], bf16,
                             kind="ExternalOutput")

    # internal DRAM
    ar_in = [nc.dram_tensor(f"ar_in{b}", [R + 2 * N, Lb], bf16)
             for b in range(B)]
    ar_out = [nc.dram_tensor(f"ar_out{b}", [R + 2 * N, Lb], bf16,
                             addr_space="Shared") for b in range(B)]
    # batch-0 ARs run as two token-half collectives (the first dispatches
    # right after the first half's dbc, overlapping second-half compute)
    arH_in = [nc.dram_tensor(f"ar0h{h}_in", [R + 2 * N, FH], bf16)
              for h in range(2)]
    arH_out = [nc.dram_tensor(f"ar0h{h}_out", [R + 2 * N, FH], bf16,
                              addr_space="Shared") for h in range(2)]
    part = [nc.dram_tensor(f"part{b}", [Lb, DM], bf16) for b in range(B)]
    rs_out = [[nc.dram_tensor(f"rs_out{b}_{q}", [HR, DM], bf16)
               for q in range(4)] for b in range(B)]
    rs1h = [nc.dram_tensor(f"rs1h{h}", [FH // N_CORES, DM], bf16)
            for h in range(2)]

    rg = [list(range(N_CORES))]

    def bcast_rows2(dram_t, row, lo, hi):
        """AP reading DRAM rows [row, row+1] cols [lo:hi) broadcast to 128
        partitions (shape [128, 2, hi-lo])."""
        sl = dram_t[row:row + 2, lo:hi]
        return _bass.AP(tensor=sl.tensor, offset=sl.offset,
                        ap=[[0, 128], list(sl.ap[0]), list(sl.ap[1])])

    with tile.TileContext(nc) as tc:
        with tc.tile_pool(name="consts", bufs=1) as consts, \
             tc.tile_pool(name="pbig", bufs=1, space="PSUM") as pbig, \
             tc.tile_pool(name="pchunk", bufs=4, space="PSUM") as pchunk, \
             tc.tile_pool(name="xt", bufs=16) as xt_pool, \
             tc.tile_pool(name="xc", bufs=2) as xc_pool, \
             tc.tile_pool(name="u", bufs=4) as u_pool, \
             tc.tile_pool(name="z", bufs=4) as z_pool, \
             tc.tile_pool(name="dbcp", bufs=1) as dbc_pool, \
             tc.tile_pool(name="dd", bufs=2) as dd_pool, \
             tc.tile_pool(name="rep", bufs=4) as rep_pool, \
             tc.tile_pool(name="sw", bufs=6) as sw_pool, \
             tc.tile_pool(name="y", bufs=3) as y_pool, \
             tc.tile_pool(name="gz", bufs=2) as gz_pool, \
             tc.tile_pool(name="st", bufs=3) as st_pool:

            # ---- constants ----
            w_inT_sb = consts.tile([128, DM // 128, 2 * E_LOC], f16)
            nc.sync.dma_start(out=w_inT_sb[:], in_=w_inT[:].rearrange(
                "(k p) m -> p k m", p=128))
            conv_diag_sb = consts.tile([128, NS, K, 128], f16)
            nc.sync.dma_start(out=conv_diag_sb[:], in_=conv_diag[:].rearrange(
                "p (s k m) -> p s k m", s=NS, k=K))
            conv_b_sb = consts.tile([128, NS], f32)
            nc.sync.dma_start(out=conv_b_sb[:], in_=conv_b[:])
            w_selT_sb = consts.tile([128, NS, R + 2 * N], f16)
            nc.sync.dma_start(out=w_selT_sb[:], in_=w_selT[:].rearrange(
                "p (s m) -> p s m", s=NS))
            dt_wT_sb = consts.tile([R, E_LOC], bf16)
            nc.sync.dma_start(out=dt_wT_sb[:], in_=dt_wT[:])
            dt_b_sb = consts.tile([128, NS], f32)
            nc.sync.dma_start(out=dt_b_sb[:], in_=dt_b[:])
            a_cols_sb = consts.tile([128, NS * N], f32)
            nc.sync.dma_start(out=a_cols_sb[:], in_=a_cols[:])
            ident_sb = consts.tile([128, 128], bf16)
            nc.sync.dma_start(out=ident_sb[:], in_=ident[:])
            d_diag_sb = consts.tile([128, NS, 128], f16)
            nc.sync.dma_start(out=d_diag_sb[:], in_=d_diag[:].rearrange(
                "p (s m) -> p s m", s=NS))
            w_outT_sb = consts.tile([128, NS, DM], f16)
            nc.sync.dma_start(out=w_outT_sb[:], in_=w_outT[:].rearrange(
                "p (s m) -> p s m", s=NS))

            u_tiles = {}
            z_tiles = {}
            zsil_tiles = {}
            y_tiles = {}

            # ================= phase 1 (per batch) =================
            def phase1(b):
                """in-proj, conv, dbc, AllReduce, z for batch b (generator:
                yields between emission chunks for interleaving)."""
                xc_tiles = {s: xc_pool.tile([128, Lb], f16, tag="xc",
                                            name=f"xc_{b}_{s}")
                            for s in range(NS)}
                for s in range(NS):
                    u_tiles[(b, s)] = u_pool.tile([128, Lb], f16, tag="u",
                                                  name=f"u_{b}_{s}")
                    z_tiles[(b, s)] = z_pool.tile([128, Lb], bf16, tag="z",
                                                  name=f"z_{b}_{s}")
                dbc_sb = dbc_pool.tile([R + 2 * N, Lb], bf16, tag="dbc",
                                       name=f"dbc_{b}")
                xt_tiles = {}

                def load_xt(fh):
                    xt_tiles[fh] = []
                    for k in range(DM // 128):
                        t = xt_pool.tile([128, FH], f16, tag="xt")
                        nc.sync.dma_start(
                            out=t[:],
                            in_=xT[k * 128:(k + 1) * 128,
                                   b * Lb + fh * FH:b * Lb + (fh + 1) * FH])
                        xt_tiles[fh].append(t)

                def mgroup(fh, m, vec_drain=False):
                    s = m % 2
                    for c in range(FH // FC):
                        pc = pchunk.tile([128, FC], f32, tag="pc",
                                         name=f"pin_{b}_{fh}_{m}_{c}")
                        for k in range(DM // 128):
                            nc.tensor.matmul(
                                pc[:],
                                lhsT=w_inT_sb[:, k, m * 128:(m + 1) * 128],
                                rhs=xt_tiles[fh][k][:, c * FC:(c + 1) * FC],
                                start=(k == 0), stop=(k == DM // 128 - 1))
                        off = fh * FH + c * FC
                        if m < 2:
                            dst = xc_tiles[s][:, off:off + FC]
                        else:
                            dst = z_tiles[(b, s)][:, off:off + FC]
                        if vec_drain:
                            # DVE drain: keeps the Scalar queue free for the
                            # AR-gated exp chain (only used in the b0 front,
                            # where the DVE is idle)
                            nc.vector.tensor_copy(dst, pc[:])
                        else:
                            nc.scalar.copy(dst, pc[:])

                def conv_chunk(s, c):
                    xc = xc_tiles[s]
                    lo = c * FC
                    pcv = pchunk.tile([128, FC], f32, tag="pc",
                                      name=f"pcv_{b}_{s}_{c}")
                    nc.tensor.matmul(
                        pcv[:], lhsT=conv_diag_sb[:, s, 2, :],
                        rhs=xc[:, lo:lo + FC], start=True, stop=False)
                    e1 = 1 if c == 0 else 0
                    nc.tensor.matmul(
                        pcv[:, e1:FC], lhsT=conv_diag_sb[:, s, 1, :],
                        rhs=xc[:, lo + e1 - 1:lo + FC - 1],
                        start=False, stop=False)
                    e2 = 2 if c == 0 else 0
                    nc.tensor.matmul(
                        pcv[:, e2:FC], lhsT=conv_diag_sb[:, s, 0, :],
                        rhs=xc[:, lo + e2 - 2:lo + FC - 2],
                        start=False, stop=True)
                    nc.scalar.activation(
                        u_tiles[(b, s)][:, lo:lo + FC], pcv[:], AF.Silu,
                        bias=conv_b_sb[:, s:s + 1])

                def dbc_chunk(c):
                    pd = pchunk.tile([R + 2 * N, FC], f32, tag="pc",
                                     name=f"pdbc_{b}_{c}")
                    for s in range(NS):
                        nc.tensor.matmul(
                            pd[:], lhsT=w_selT_sb[:, s, :],
                            rhs=u_tiles[(b, s)][:, c * FC:(c + 1) * FC],
                            start=(s == 0), stop=(s == NS - 1))
                    cs = slice(c * FC, (c + 1) * FC)
                    nc.scalar.copy(dbc_sb[:, cs], pd[:])
                    if b == 0:
                        nc.sync.dma_start(
                            out=arH_in[c // 2][:, (c % 2) * FC:
                                               (c % 2 + 1) * FC],
                            in_=dbc_sb[:, cs])
                    else:
                        nc.sync.dma_start(out=ar_in[b][:, cs],
                                          in_=dbc_sb[:, cs])

                if b == 0:
                    # conv/dbc per token-half so the half-AR dispatches as
                    # early as possible; both xT halves stay resident so the
                    # z m-groups after the ARs need NO new DMA traffic (they
                    # fill the PE bubble while the ARs run, without
                    # polluting DRAM bandwidth the collectives need)
                    load_xt(0)
                    load_xt(1)
                    yield
                    for fh in range(2):
                        for m in range(2):
                            mgroup(fh, m)
                            yield
                        for s in range(NS):
                            for c in range(fh * 2, fh * 2 + 2):
                                conv_chunk(s, c)
                            yield
                        for c in range(fh * 2, fh * 2 + 2):
                            dbc_chunk(c)
                        # per-half AllReduce: the first dispatches early
                        nc.gpsimd.collective_compute(
                            "AllReduce", OP.add, replica_groups=rg,
                            ins=[arH_in[fh][:]], outs=[arH_out[fh][:]])
                        yield
                    for fh in range(2):
                        for m in range(2, 4):
                            mgroup(fh, m, vec_drain=True)
                            yield
                    # silu(z) here shares the front's Silu act table and
                    # hides in the AR bubble (no mid-phase table switch)
                    for s in range(NS):
                        zsil_tiles[(b, s)] = gz_pool.tile(
                            [128, Lb], bf16, tag="sg", name=f"zsil_{b}_{s}")
                        nc.scalar.activation(zsil_tiles[(b, s)][:],
                                             z_tiles[(b, s)][:], AF.Silu)
                else:
                    # batch 1 is not latency-critical: emit all in-proj
                    # m-groups first (their Scalar side is cheap same-table
                    # copies, so they can pre-run without delaying the
                    # batch-0 exp chain), convs/dbc later
                    load_xt(0)
                    yield
                    mgroup(0, 0)
                    yield
                    mgroup(0, 1)
                    load_xt(1)
                    yield
                    mgroup(1, 0)
                    yield
                    mgroup(1, 1)
                    yield
                    for s in range(NS):
                        for c in range(4):
                            conv_chunk(s, c)
                        yield
                    for c in range(4):
                        dbc_chunk(c)
                    nc.gpsimd.collective_compute(
                        "AllReduce", OP.add, replica_groups=rg,
                        ins=[ar_in[b][:]], outs=[ar_out[b][:]])
                    yield
                    for fh in range(2):
                        load_xt(fh)      # reload (xt pool too small)
                        yield
                        for m in range(2, 4):
                            mgroup(fh, m)
                            yield

            # ================= phase 2 =================
            def prep_batch_alloc(b):
                return dd_pool.tile([R, Lb], bf16, tag="dtlow",
                                    name=f"dtlow_{b}")

            def load_dtlow(b, dtlow, h):
                """One dt_low half from the AllReduce output (emitted as
                late as possible: this DMA waits on the AR, and everything
                behind it in the Sync queue head-of-line blocks)."""
                cs = slice(h * FH, (h + 1) * FH)
                if b == 0:
                    nc.sync.dma_start(out=dtlow[:, cs],
                                      in_=arH_out[h][0:R, :])
                else:
                    nc.sync.dma_start(out=dtlow[:, cs],
                                      in_=ar_out[b][0:R, cs])

            def prep_alloc(b, s):
                delta = dd_pool.tile([128, Lb], f16, tag="delta",
                                     name=f"delta_{b}_{s}")
                du = dd_pool.tile([128, Lb], f16, tag="du",
                                  name=f"du_{b}_{s}")
                return delta, du

            def prep_half(b, s, dtlow, delta, du, h):
                """delta (softplus) and du for (b, s), token half h."""
                ets = []
                for c in range(2 * h, 2 * h + 2):
                    pd = pchunk.tile([128, FC], f32, tag="pc",
                                     name=f"pdt_{b}_{s}_{c}")
                    nc.tensor.matmul(
                        pd[:], lhsT=dt_wT_sb[:, s * 128:(s + 1) * 128],
                        rhs=dtlow[:, c * FC:(c + 1) * FC],
                        start=True, stop=True)
                    et = st_pool.tile([128, FC], f32, tag="st",
                                      name=f"et_{b}_{s}_{c}")
                    nc.scalar.activation(et[:], pd[:], AF.Exp,
                                         bias=dt_b_sb[:, s:s + 1])
                    ets.append((c, et))
                for c, et in ets:
                    nc.scalar.activation(delta[:, c * FC:(c + 1) * FC],
                                         et, AF.Ln, bias=1.0)
                cs = slice(h * FH, (h + 1) * FH)
                nc.vector.tensor_mul(du[:, cs], delta[:, cs],
                                     u_tiles[(b, s)][:, cs])

            def scan_s(b, s, delta, du, bg=None, segs=None, srcs=None,
                       seg_hook=None):
                """16-state scan for (b, s); returns the open PSUM y tile.
                bg: generator stepped between n-iterations.
                segs: chained token segments [(lo, hi), ...]; srcs: per-seg
                (dram_tensor, col_offset) for the B/C broadcast rows.
                seg_hook(si, lo, hi, py) runs after a segment's y (incl.
                skip term) is accumulated."""
                if segs is None:
                    segs = [(0, Lb)]
                if srcs is None:
                    if b == 0:
                        srcs = [[(arH_out[0], 0, 0, FH),
                                 (arH_out[1], FH, FH, Lb)]] * len(segs)
                    else:
                        srcs = [[(ar_out[b], 0, 0, Lb)]] * len(segs)
                py = pbig.tile([128, Lb], f32, tag="pbig", name=f"py_{b}_{s}")
                carry = None
                if len(segs) > 1:
                    carry = dd_pool.tile([128, N], f32, tag="carry",
                                         name=f"carry_{b}_{s}")
                reps = {}
                order = [(si, n) for si in range(len(segs)) for n in range(N)]

                def fetch(i):
                    if i >= len(order):
                        return
                    si, n = order[i]
                    lo, hi = segs[si]
                    bc = rep_pool.tile([128, 2, Lb], bf16, tag="rep",
                                       name=f"bc_{b}_{s}_{lo}_{n}")
                    for src_t, t_off, plo, phi in srcs[si]:
                        plo, phi = max(plo, lo), min(phi, hi)
                        if phi <= plo:
                            continue
                        nc.sync.dma_start(
                            out=bc[:, :, plo - lo:phi - lo],
                            in_=bcast_rows2(src_t, R + 2 * n,
                                            plo - t_off, phi - t_off))
                    reps[(si, n)] = bc

                fetch(0)
                fetch(1)
                fetch(2)
                for si, (lo, hi) in enumerate(segs):
                    w = hi - lo
                    for n in range(N):
                        fetch(si * N + n + 3)
                        bc = reps.pop((si, n))
                        br = bc[:, 0, :]
                        cr = bc[:, 1, :]
                        a_sb = sw_pool.tile([128, Lb], bf16, tag="sw",
                                            name=f"a_{b}_{s}_{lo}_{n}")
                        nc.scalar.activation(
                            a_sb[:, 0:w], delta[:, lo:hi], AF.Exp,
                            scale=a_cols_sb[:, s * N + n:s * N + n + 1])
                        b_sb = sw_pool.tile([128, Lb], bf16, tag="sw",
                                            name=f"b_{b}_{s}_{lo}_{n}")
                        nc.vector.tensor_mul(b_sb[:, 0:w], du[:, lo:hi],
                                             br[:, 0:w])
                        h_sb = sw_pool.tile([128, Lb], bf16, tag="sw",
                                            name=f"h_{b}_{s}_{lo}_{n}")
                        init = 0.0 if si == 0 else carry[:, n:n + 1]
                        nc.vector.tensor_tensor_scan(
                            h_sb[:, 0:w], a_sb[:, 0:w], b_sb[:, 0:w], init,
                            op0=OP.mult, op1=OP.add)
                        if si + 1 < len(segs):
                            nc.vector.tensor_copy(carry[:, n:n + 1],
                                                  h_sb[:, w - 1:w])
                        hc_sb = sw_pool.tile([128, Lb], bf16, tag="sw",
                                             name=f"hc_{b}_{s}_{lo}_{n}")
                        nc.vector.tensor_mul(hc_sb[:, 0:w], h_sb[:, 0:w],
                                             cr[:, 0:w])
                        for c in range(lo // FC, hi // FC):
                            nc.tensor.matmul(
                                py[:, c * FC:(c + 1) * FC], lhsT=ident_sb[:],
                                rhs=hc_sb[:, c * FC - lo:(c + 1) * FC - lo],
                                start=(n == 0), stop=False)
                        if bg is not None:
                            next(bg, None)
                    # skip term: py += diag(D) @ u for this segment
                    for c in range(lo // FC, hi // FC):
                        nc.tensor.matmul(
                            py[:, c * FC:(c + 1) * FC],
                            lhsT=d_diag_sb[:, s, :],
                            rhs=u_tiles[(b, s)][:, c * FC:(c + 1) * FC],
                            start=False, stop=True)
                    if seg_hook is not None:
                        seg_hook(si, lo, hi, py)
                return py

            yd_tiles = {}

            def yasm_drain(b, s, py, lo=0, hi=None):
                """Drain PSUM y [lo:hi) to bf16 (Scalar).  Must be emitted
                BEFORE the next scan group's py allocation: it is the only
                reader of the single rotating PSUM buffer, and a reader
                emitted after the next writer is a race."""
                hi = Lb if hi is None else hi
                if (b, s) not in yd_tiles:
                    yd_tiles[(b, s)] = gz_pool.tile(
                        [128, Lb], bf16, tag="yd", name=f"yd_{b}_{s}")
                nc.scalar.copy(yd_tiles[(b, s)][:, lo:hi], py[:, lo:hi])

            def yasm_gate(b, s, lo=0, hi=None):
                """Gate the drained y with silu(z) (one DVE mul); safe to
                emit inside the next scan's background."""
                hi = Lb if hi is None else hi
                if (b, s) not in y_tiles:
                    y_tiles[(b, s)] = y_pool.tile([128, Lb], f16, tag="y",
                                                  name=f"yg_{b}_{s}")
                if (b, s) not in zsil_tiles:
                    zsil_tiles[(b, s)] = gz_pool.tile(
                        [128, Lb], bf16, tag="sg", name=f"zsil_{b}_{s}")
                    nc.scalar.activation(zsil_tiles[(b, s)][:],
                                         z_tiles[(b, s)][:], AF.Silu)
                nc.vector.tensor_mul(y_tiles[(b, s)][:, lo:hi],
                                     yd_tiles[(b, s)][:, lo:hi],
                                     zsil_tiles[(b, s)][:, lo:hi])

            def yasm(b, s, py, lo=0, hi=None):
                yasm_drain(b, s, py, lo, hi)
                yasm_gate(b, s, lo, hi)

            def outproj_mt(b, mt):
                """One 128-token out-proj block: po chunks -> part DRAM."""
                for f in range(DM // FC):
                    po = pchunk.tile([128, FC], f32, tag="pc",
                                     name=f"po_{b}_{mt}_{f}")
                    for s in range(NS):
                        nc.tensor.matmul(
                            po[:],
                            lhsT=y_tiles[(b, s)][:, mt * 128:(mt + 1) * 128],
                            rhs=w_outT_sb[:, s, f * FC:(f + 1) * FC],
                            start=(s == 0), stop=(s == NS - 1))
                    sto = st_pool.tile([128, FC], bf16, tag="st",
                                       name=f"sto_{b}_{mt}_{f}")
                    nc.scalar.copy(sto[:], po[:])
                    nc.sync.dma_start(
                        out=part[b][mt * 128:(mt + 1) * 128,
                                    f * FC:(f + 1) * FC],
                        in_=sto[:])

            def rs_quarter(b, q):
                nc.gpsimd.collective_compute(
                    "ReduceScatter", OP.add, replica_groups=rg,
                    ins=[part[b][q * QT:(q + 1) * QT, :]],
                    outs=[rs_out[b][q][:]])
                nc.sync.dma_start(
                    out=out_loc[(b * 4 + q) * HR:(b * 4 + q + 1) * HR, :],
                    in_=rs_out[b][q][:])

            def outproj0():
                """Batch-0 out-proj with quarter ReduceScatters
                (generator)."""
                for mt in range(Lb // 128):
                    outproj_mt(0, mt)
                    yield
                    if mt % 4 == 3:
                        rs_quarter(0, mt // 4)
                        yield

            def run_gen(g):
                for _ in g:
                    pass

            # --------- emission schedule ---------
            def chain(*gens):
                for g in gens:
                    yield from g

            def gen_call(fn, *args):
                fn(*args)
                yield

            def skip(k):
                for _ in range(k):
                    yield

            run_gen(phase1(0))           # entire b0 front (z fills the
            dtlow0 = prep_batch_alloc(0)  # PE bubble while the ARs run)
            load_dtlow(0, dtlow0, 0)
            d0, du0 = prep_alloc(0, 0)
            prep_half(0, 0, dtlow0, d0, du0, 0)
            d1, du1 = prep_alloc(0, 1)
            g_rest = phase1(1)           # all of b1 as scan background
            # prep work for the NEXT scan group is threaded into the
            # current scan's background so group boundaries have no
            # Scalar-chain stall
            bg00 = chain(islice(g_rest, 2),
                         gen_call(load_dtlow, 0, dtlow0, 1),
                         islice(g_rest, 1),
                         gen_call(prep_half, 0, 0, dtlow0, d0, du0, 1),
                         islice(g_rest, 7),
                         gen_call(prep_half, 0, 1, dtlow0, d1, du1, 0),
                         islice(g_rest, 3),
                         gen_call(prep_half, 0, 1, dtlow0, d1, du1, 1),
                         g_rest)
            py00 = scan_s(0, 0, d0, du0, bg=bg00,
                          segs=[(0, FH), (FH, Lb)])
            yasm_drain(0, 0, py00)
            dtlow1 = prep_batch_alloc(1)
            d2, du2 = prep_alloc(1, 0)
            bg01 = chain(gen_call(yasm_gate, 0, 0), skip(6),
                         gen_call(load_dtlow, 1, dtlow1, 0),
                         gen_call(load_dtlow, 1, dtlow1, 1),
                         gen_call(prep_half, 1, 0, dtlow1, d2, du2, 0),
                         gen_call(prep_half, 1, 0, dtlow1, d2, du2, 1),
                         g_rest)
            py = scan_s(0, 1, d1, du1, bg=bg01)
            yasm_drain(0, 1, py)
            d3, du3 = prep_alloc(1, 1)
            g_op0 = outproj0()
            g_tail = chain(gen_call(yasm_gate, 0, 1), skip(4),
                           gen_call(prep_half, 1, 1, dtlow1, d3, du3, 0),
                           gen_call(prep_half, 1, 1, dtlow1, d3, du3, 1),
                           g_op0)
            py = scan_s(1, 0, d2, du2, bg=g_tail)
            yasm_drain(1, 0, py)
            g_tail2 = chain(gen_call(yasm_gate, 1, 0), g_tail)
            py = scan_s(1, 1, d3, du3, bg=g_tail2)
            run_gen(g_tail2)
            # serial 2-half tail: half-1's drain/out-proj compute overlaps
            # half-0's ReduceScatter (no scans left to starve)
            hrh = FH // N_CORES
            for h in range(2):
                lo, hi = h * FH, (h + 1) * FH
                yasm(1, 1, py, lo, hi)
                for mt in range(lo // 128, hi // 128):
                    outproj_mt(1, mt)
                nc.gpsimd.collective_compute(
                    "ReduceScatter", OP.add, replica_groups=rg,
                    ins=[part[1][lo:hi, :]], outs=[rs1h[h][:]])
                nc.sync.dma_start(
                    out=out_loc[4 * HR + h * hrh:4 * HR + (h + 1) * hrh, :],
                    in_=rs1h[h][:])

    nc.compile()
    _PROGRAM_CACHE[key] = nc
    return nc


# ---------------------------------------------------------------------------
def host_prep(inputs, Lb=L):
    x = np.asarray(inputs["x"], np.float32)
    W_in = np.asarray(inputs["W_in"], np.float32)
    conv_w = np.asarray(inputs["conv_w"], np.float32)
    conv_b = np.asarray(inputs["conv_b"], np.float32)
    W_sel = np.asarray(inputs["W_sel"], np.float32)
    dt_w = np.asarray(inputs["dt_w"], np.float32)
    dt_b = np.asarray(inputs["dt_b"], np.float32)
    A_log = np.asarray(inputs["A_log"], np.float32)
    D_param = np.asarray(inputs["D_param"], np.float32)
    W_out = np.asarray(inputs["W_out"], np.float32)

    import ml_dtypes
    bf16 = ml_dtypes.bfloat16
    tok = B * Lb
    xT = np.ascontiguousarray(
        x[:, :Lb, :].reshape(tok, DM).T).astype(np.float16)
    A = -np.exp(A_log.astype(np.float64)).astype(np.float32)   # [E, N]

    # permute W_sel rows so B_n and C_n are adjacent (rows R+2n, R+2n+1):
    # one broadcast DMA then fetches both rows per state
    perm = list(range(R))
    for n in range(N):
        perm += [R + n, R + N + n]
    W_sel = W_sel[perm, :]

    ident = np.eye(128, dtype=np.float32)

    in_maps = []
    for k in range(N_CORES):
        es = slice(k * E_LOC, (k + 1) * E_LOC)
        W_in_loc = np.concatenate([W_in[k * E_LOC:(k + 1) * E_LOC],
                                   W_in[E + k * E_LOC:E + (k + 1) * E_LOC]],
                                  axis=0)            # [2*E_LOC, DM]
        A_loc = A[es]                                # [E_LOC, N]

        # a_cols[p, s*N + n] = A_loc[s*128+p, n]
        a_cols = np.zeros((128, NS * N), np.float32)
        for s in range(NS):
            for n in range(N):
                a_cols[:, s * N + n] = A_loc[s * 128:(s + 1) * 128, n]

        # conv_diag[p, s, kk, :] = diag of conv_w[es][s*128+p] tap kk
        conv_diag = np.zeros((128, NS, K, 128), np.float32)
        for s in range(NS):
            for kk in range(K):
                conv_diag[:, s, kk, :] = np.diag(
                    conv_w[es][s * 128:(s + 1) * 128, 0, kk])
        d_diag = np.zeros((128, NS, 128), np.float32)
        for s in range(NS):
            d_diag[:, s, :] = np.diag(D_param[es][s * 128:(s + 1) * 128])

        def two(v):  # [E_LOC] -> [128, NS]
            return np.ascontiguousarray(v.reshape(NS, 128).T)

        in_maps.append({
            "xT": xT,
            "w_inT": np.ascontiguousarray(W_in_loc.T).astype(np.float16),
            "conv_diag": np.ascontiguousarray(
                conv_diag.reshape(128, NS * K * 128)).astype(np.float16),
            "conv_b": two(conv_b[es]),
            "w_selT": np.ascontiguousarray(
                W_sel[:, es].T.reshape(NS, 128, R + 2 * N).transpose(
                    1, 0, 2).reshape(128, NS * (R + 2 * N))).astype(
                        np.float16),
            "dt_wT": np.ascontiguousarray(dt_w[es].T).astype(bf16),
            "dt_b": two(dt_b[es]),
            "a_cols": a_cols,
            "ident": ident.astype(bf16),
            "d_diag": np.ascontiguousarray(
                d_diag.reshape(128, NS * 128)).astype(np.float16),
            "w_outT": np.ascontiguousarray(
                W_out[:, es].T.reshape(NS, 128, DM).transpose(
                    1, 0, 2).reshape(128, NS * DM)).astype(np.float16),
        })
    return in_maps


def assemble_output(results, Lb=L):
    out = np.empty((B, Lb, DM), np.float32)
    QT = Lb // 4
    hr = QT // N_CORES
    lr = Lb // N_CORES
    for c in range(N_CORES):
        chunk = np.asarray(results[c]["out_loc"], np.float32)
        for q in range(4):       # batch 0: quarter ReduceScatters
            out[0, q * QT + c * hr:q * QT + (c + 1) * hr, :] = \
                chunk[q * hr:(q + 1) * hr, :]
        # batch 1: two half-sequence ReduceScatters
        hh = (Lb // 2) // N_CORES
        for h in range(2):
            out[1, h * (Lb // 2) + c * hh:h * (Lb // 2) + (c + 1) * hh, :] \
                = chunk[4 * hr + h * hh:4 * hr + (h + 1) * hh, :]
    return out


def kernel(**inputs) -> np.ndarray:
    from concourse import bass_utils
    nc = build_program()
    in_maps = host_prep(inputs)
    res = bass_utils.run_bass_kernel_spmd(nc, in_maps, list(range(N_CORES)))
    return assemble_output(res.results).astype(np.float32)
